# revision 5
# baseline (speedup 1.0000x reference)
"""MoE layer (top-2 routing, 8 experts) on 8 Trainium2 NeuronCores.

Strategy (expert-parallel, per sharding hint):
  - Host computes the gate (replicated router math in fp64 numpy): logits,
    top-2 experts per token, softmax gates.
  - Tokens are dispatched (host-side all-to-all) into per-expert batches,
    padded to a common capacity; core c holds expert c's weights and its
    token batch.
  - Each core runs the expert MLP: Y^T = g ⊙ (relu(W1^T X^T + b1) W2)^T in
    a feature-major (transposed) dataflow, bf16 matmuls with fp32 PSUM
    accumulation.
  - Host combines: out[tok] += Y rows (+ g * b2), summing each token's two
    expert contributions.

Hardcoded problem shape: x(8192,1024) w1(8,1024,4096) w2(8,4096,1024).
"""

import numpy as np
import ml_dtypes

import concourse.bass as bass  # noqa: F401  (bass types referenced via tile/bacc)
import concourse.tile as tile
import concourse.mybir as mybir
from concourse import bacc
from concourse.bass_utils import run_bass_kernel_spmd

E = 8          # experts == cores
D = 1024       # model dim
H = 4096       # hidden dim
TOP_K = 2
N_CORES = 8
ND = D // 128  # 8 d-tiles
NH = H // 128  # 32 h-tiles

F32 = mybir.dt.float32
F16 = mybir.dt.float16


def _token_tiles(cap):
    tiles = []
    t = 0
    while t < cap:
        n = min(512, cap - t)
        tiles.append((t, n))
        t += n
    return tiles


def build_moe(cap):
    """Build + compile the per-core expert-MLP Bass program for capacity cap."""
    nc = bacc.Bacc("TRN2", target_bir_lowering=False, debug=False, num_devices=N_CORES)

    xt = nc.dram_tensor("xt", [D, cap], F16, kind="ExternalInput")      # x[idx].T
    w1 = nc.dram_tensor("w1", [D, H], F16, kind="ExternalInput")
    w2 = nc.dram_tensor("w2", [H, D], F16, kind="ExternalInput")
    b1 = nc.dram_tensor("b1", [128, NH], F32, kind="ExternalInput")      # b1[p,j]=b1_full[j*128+p]
    g = nc.dram_tensor("g", [128, cap], F32, kind="ExternalInput")       # gate, replicated rows
    yt = nc.dram_tensor("yt", [D, cap], F32, kind="ExternalOutput")

    xt_ap, w1_ap, w2_ap, b1_ap, g_ap, yt_ap = (
        t.ap() for t in (xt, w1, w2, b1, g, yt)
    )

    with tile.TileContext(nc) as tc:
        with (
            tc.tile_pool(name="wpool", bufs=1) as wpool,
            tc.tile_pool(name="xpool", bufs=16) as xpool,
            tc.tile_pool(name="hpool", bufs=36) as hpool,
            tc.tile_pool(name="ypool", bufs=4) as ypool,
            tc.tile_pool(name="gpool", bufs=2) as gpool,
            tc.tile_pool(name="ph", bufs=4, space="PSUM") as ph_pool,
            tc.tile_pool(name="py", bufs=2, space="PSUM") as py_pool,
        ):
            def load_tok_tile(t0, tn):
                g_sb = gpool.tile([128, 512], F32, name=f"gsb{t0}", tag="gsb")
                nc.sync.dma_start(g_sb[:, :tn], g_ap[:, t0:t0 + tn])
                x_sb = []
                for d in range(ND):
                    t = xpool.tile([128, 512], F16, name=f"xsb{t0}_{d}", tag="xsb")
                    nc.sync.dma_start(t[:, :tn], xt_ap[d * 128:(d + 1) * 128, t0:t0 + tn])
                    x_sb.append(t)
                return g_sb, x_sb

            tiles = _token_tiles(cap)
            # Prefetch tile 0's activations BEFORE the bulk weight DMAs so the
            # first layer-1 matmuls aren't queued behind 16.8MB of weights.
            prefetched = {tiles[0][0]: load_tok_tile(*tiles[0])}

            # Resident weights. w1 is loaded as 256 [128,128] tiles in exact
            # layer-1 first-consumption order (h-major) so tile 0's matmuls
            # get their weights just-in-time instead of waiting on 1MB slices.
            w1_sb = [[None] * NH for _ in range(ND)]
            for h in range(NH):
                for d in range(ND):
                    t = wpool.tile([128, 128], F16, name=f"w1t{d}_{h}", tag=f"w1t{d}_{h}")
                    nc.sync.dma_start(
                        t[:], w1_ap[d * 128:(d + 1) * 128, h * 128:(h + 1) * 128]
                    )
                    w1_sb[d][h] = t
            b1_sb = wpool.tile([128, NH], F32, name="b1sb", tag="b1sb")
            nc.sync.dma_start(b1_sb[:], b1_ap[:, :])
            w2_sb = []
            for h in range(NH):
                t = wpool.tile([128, D], F16, name=f"w2sb{h}", tag=f"w2sb{h}")
                nc.sync.dma_start(t[:], w2_ap[h * 128:(h + 1) * 128, :])
                w2_sb.append(t)

            for (t0, tn) in tiles:
                g_sb, x_sb = prefetched.pop(t0) if t0 in prefetched else load_tok_tile(t0, tn)

                # Layer 1: H^T[h_tile] = relu(sum_d W1[d,h]^T X^T[d] + b1)
                h_sb = []
                for h in range(NH):
                    ph = ph_pool.tile([128, 512], F32, name=f"ph{t0}_{h}", tag="ph")
                    for d in range(ND):
                        nc.tensor.matmul(
                            ph[:, :tn],
                            w1_sb[d][h][:],
                            x_sb[d][:, :tn],
                            start=(d == 0),
                            stop=(d == ND - 1),
                        )
                    ht = hpool.tile([128, 512], F16, name=f"hsb{t0}_{h}", tag="hsb")
                    nc.scalar.activation(
                        ht[:, :tn], ph[:, :tn],
                        mybir.ActivationFunctionType.Relu,
                        bias=b1_sb[:, h:h + 1],
                    )
                    h_sb.append(ht)

                # Layer 2: Y^T[do] = g ⊙ sum_h W2[h,do]^T H^T[h]
                for do in range(ND):
                    py = py_pool.tile([128, 512], F32, name=f"py{t0}_{do}", tag="py")
                    for h in range(NH):
                        nc.tensor.matmul(
                            py[:, :tn],
                            w2_sb[h][:, do * 128:(do + 1) * 128],
                            h_sb[h][:, :tn],
                            start=(h == 0),
                            stop=(h == NH - 1),
                        )
                    y_sb = ypool.tile([128, 512], F32, name=f"ysb{t0}_{do}", tag="ysb")
                    nc.vector.tensor_mul(y_sb[:, :tn], py[:, :tn], g_sb[:, :tn])
                    nc.sync.dma_start(yt_ap[do * 128:(do + 1) * 128, t0:t0 + tn], y_sb[:, :tn])

    nc.compile()
    return nc


def _route(x, wg, bg):
    """Host router in fp64: per-token top-2 experts and softmax gates."""
    logits = x.astype(np.float64) @ wg.astype(np.float64).T + bg.astype(np.float64)
    top2 = np.argpartition(-logits, 1, axis=1)[:, :TOP_K]  # two largest, unordered
    vals = np.take_along_axis(logits, top2, axis=1)
    ex = np.exp(vals - vals.max(axis=1, keepdims=True))
    gates = ex / ex.sum(axis=1, keepdims=True)
    idxs, gs = [], []
    for e in range(E):
        mask = top2 == e
        rows = np.nonzero(mask.any(axis=1))[0]
        idxs.append(rows)
        gs.append(gates[mask].astype(np.float32))
    return idxs, gs


def moe_run(x, wg, bg, w1, b1, w2, b2, trace=False, trace_kwargs=None):
    x = np.ascontiguousarray(np.asarray(x, np.float32))
    wg = np.asarray(wg, np.float32)
    bg = np.asarray(bg, np.float32)
    w1 = np.asarray(w1, np.float32)
    b1 = np.asarray(b1, np.float32)
    w2 = np.asarray(w2, np.float32)
    b2 = np.asarray(b2, np.float32)
    B = x.shape[0]

    idxs, gs = _route(x, wg, bg)
    cap = max(256, -(-max(len(r) for r in idxs) // 128) * 128)

    nc = build_moe(cap)

    in_maps = []
    for e in range(E):
        n = len(idxs[e])
        xe = np.zeros((cap, D), np.float32)
        xe[:n] = x[idxs[e]]
        ge = np.zeros((cap,), np.float32)
        ge[:n] = gs[e]
        in_maps.append({
            "xt": np.ascontiguousarray(xe.T).astype(np.float16),
            "w1": w1[e].astype(np.float16),
            "w2": w2[e].astype(np.float16),
            "b1": np.ascontiguousarray(b1[e].reshape(NH, 128).T),
            "g": np.ascontiguousarray(np.broadcast_to(ge, (128, cap))),
        })

    kwargs = {}
    if trace:
        kwargs["trace"] = True
        if trace_kwargs:
            kwargs.update(trace_kwargs)
    res = run_bass_kernel_spmd(nc, in_maps, core_ids=list(range(N_CORES)), **kwargs)

    out = np.zeros((B, D), np.float32)
    for e in range(E):
        n = len(idxs[e])
        y = res.results[e]["yt"][:, :n].T  # (n, D), gate already applied
        out[idxs[e]] += y + gs[e][:, None] * b2[e][None, :]
    return out, res


def kernel(x, wg, bg, w1, b1, w2, b2):
    out, _ = moe_run(x, wg, bg, w1, b1, w2, b2, trace=False)
    return out


# revision 7
# speedup vs baseline: 1.2704x; 1.2704x over previous
"""MoE layer (top-2 routing, 8 experts) on 8 Trainium2 NeuronCores.

Strategy (expert-parallel, per sharding hint):
  - Host computes the gate (replicated router math in fp64 numpy): logits,
    top-2 experts per token, softmax gates.
  - Tokens are dispatched (host-side all-to-all) into per-expert batches,
    padded to a common capacity; core c holds expert c's weights and its
    token batch.
  - Each core runs the expert MLP: Y^T = g ⊙ (relu(W1^T X^T + b1) W2)^T in
    a feature-major (transposed) dataflow, bf16 matmuls with fp32 PSUM
    accumulation.
  - Host combines: out[tok] += Y rows (+ g * b2), summing each token's two
    expert contributions.

Hardcoded problem shape: x(8192,1024) w1(8,1024,4096) w2(8,4096,1024).
"""

import numpy as np
import ml_dtypes

import concourse.bass as bass  # noqa: F401  (bass types referenced via tile/bacc)
import concourse.tile as tile
import concourse.mybir as mybir
from concourse import bacc
from concourse.bass_utils import run_bass_kernel_spmd

E = 8          # experts == cores
D = 1024       # model dim
H = 4096       # hidden dim
TOP_K = 2
N_CORES = 8
ND = D // 128  # 8 d-tiles
NH = H // 128  # 32 h-tiles

F32 = mybir.dt.float32
F16 = mybir.dt.float16


def _token_tiles(cap):
    tiles = []
    t = 0
    while t < cap:
        n = min(512, cap - t)
        tiles.append((t, n))
        t += n
    return tiles


def build_moe(cap):
    """Build + compile the per-core expert-MLP Bass program for capacity cap."""
    nc = bacc.Bacc("TRN2", target_bir_lowering=False, debug=False, num_devices=N_CORES)

    xt = nc.dram_tensor("xt", [D, cap], F16, kind="ExternalInput")      # x[idx].T
    w1 = nc.dram_tensor("w1", [D, H], F16, kind="ExternalInput")
    w2 = nc.dram_tensor("w2", [H, D], F16, kind="ExternalInput")
    b1 = nc.dram_tensor("b1", [128, NH], F32, kind="ExternalInput")      # b1[p,j]=b1_full[j*128+p]
    g = nc.dram_tensor("g", [128, cap], F32, kind="ExternalInput")       # gate, replicated rows
    yt = nc.dram_tensor("yt", [D, cap], F32, kind="ExternalOutput")

    xt_ap, w1_ap, w2_ap, b1_ap, g_ap, yt_ap = (
        t.ap() for t in (xt, w1, w2, b1, g, yt)
    )

    with tile.TileContext(nc) as tc:
        with (
            tc.tile_pool(name="wpool", bufs=1) as wpool,
            tc.tile_pool(name="xpool", bufs=16) as xpool,
            tc.tile_pool(name="hpool", bufs=36) as hpool,
            tc.tile_pool(name="ypool", bufs=4) as ypool,
            tc.tile_pool(name="gpool", bufs=2) as gpool,
            tc.tile_pool(name="ph", bufs=4, space="PSUM") as ph_pool,
            tc.tile_pool(name="py", bufs=2, space="PSUM") as py_pool,
        ):
            def load_tok_tile(t0, tn):
                g_sb = gpool.tile([128, 512], F32, name=f"gsb{t0}", tag="gsb")
                nc.sync.dma_start(g_sb[:, :tn], g_ap[:, t0:t0 + tn])
                x_sb = []
                for d in range(ND):
                    t = xpool.tile([128, 512], F16, name=f"xsb{t0}_{d}", tag="xsb")
                    nc.sync.dma_start(t[:, :tn], xt_ap[d * 128:(d + 1) * 128, t0:t0 + tn])
                    x_sb.append(t)
                return g_sb, x_sb

            tiles = _token_tiles(cap)
            # Prefetch tile 0's activations BEFORE the bulk weight DMAs so the
            # first layer-1 matmuls aren't queued behind 16.8MB of weights.
            prefetched = {tiles[0][0]: load_tok_tile(*tiles[0])}

            # Resident weights. w1 is loaded as [128, 1024] chunks (8 h-tiles
            # per chunk, 2KB DMA lines) in layer-1 consumption order so tile
            # 0's first matmul group only waits on ~2MB, not all of w1.
            HC = 1024
            n_hc = H // HC
            w1_sb = [[None] * n_hc for _ in range(ND)]
            for hc in range(n_hc):
                for d in range(ND):
                    t = wpool.tile([128, HC], F16, name=f"w1c{d}_{hc}", tag=f"w1c{d}_{hc}")
                    nc.sync.dma_start(
                        t[:], w1_ap[d * 128:(d + 1) * 128, hc * HC:(hc + 1) * HC]
                    )
                    w1_sb[d][hc] = t
            b1_sb = wpool.tile([128, NH], F32, name="b1sb", tag="b1sb")
            nc.sync.dma_start(b1_sb[:], b1_ap[:, :])
            w2_sb = []
            for h in range(NH):
                t = wpool.tile([128, D], F16, name=f"w2sb{h}", tag=f"w2sb{h}")
                nc.sync.dma_start(t[:], w2_ap[h * 128:(h + 1) * 128, :])
                w2_sb.append(t)

            for (t0, tn) in tiles:
                g_sb, x_sb = prefetched.pop(t0) if t0 in prefetched else load_tok_tile(t0, tn)

                # Layer 1: H^T[h_tile] = relu(sum_d W1[d,h]^T X^T[d] + b1)
                h_sb = []
                for h in range(NH):
                    ph = ph_pool.tile([128, 512], F32, name=f"ph{t0}_{h}", tag="ph")
                    hc, ho = divmod(h * 128, HC)
                    for d in range(ND):
                        nc.tensor.matmul(
                            ph[:, :tn],
                            w1_sb[d][hc][:, ho:ho + 128],
                            x_sb[d][:, :tn],
                            start=(d == 0),
                            stop=(d == ND - 1),
                        )
                    ht = hpool.tile([128, 512], F16, name=f"hsb{t0}_{h}", tag="hsb")
                    nc.scalar.activation(
                        ht[:, :tn], ph[:, :tn],
                        mybir.ActivationFunctionType.Relu,
                        bias=b1_sb[:, h:h + 1],
                    )
                    h_sb.append(ht)

                # Layer 2: Y^T[do] = g ⊙ sum_h W2[h,do]^T H^T[h]
                for do in range(ND):
                    py = py_pool.tile([128, 512], F32, name=f"py{t0}_{do}", tag="py")
                    for h in range(NH):
                        nc.tensor.matmul(
                            py[:, :tn],
                            w2_sb[h][:, do * 128:(do + 1) * 128],
                            h_sb[h][:, :tn],
                            start=(h == 0),
                            stop=(h == NH - 1),
                        )
                    y_sb = ypool.tile([128, 512], F32, name=f"ysb{t0}_{do}", tag="ysb")
                    nc.vector.tensor_mul(y_sb[:, :tn], py[:, :tn], g_sb[:, :tn])
                    nc.sync.dma_start(yt_ap[do * 128:(do + 1) * 128, t0:t0 + tn], y_sb[:, :tn])

    nc.compile()
    return nc


def _route(x, wg, bg):
    """Host router in fp64: per-token top-2 experts and softmax gates."""
    logits = x.astype(np.float64) @ wg.astype(np.float64).T + bg.astype(np.float64)
    top2 = np.argpartition(-logits, 1, axis=1)[:, :TOP_K]  # two largest, unordered
    vals = np.take_along_axis(logits, top2, axis=1)
    ex = np.exp(vals - vals.max(axis=1, keepdims=True))
    gates = ex / ex.sum(axis=1, keepdims=True)
    idxs, gs = [], []
    for e in range(E):
        mask = top2 == e
        rows = np.nonzero(mask.any(axis=1))[0]
        idxs.append(rows)
        gs.append(gates[mask].astype(np.float32))
    return idxs, gs


def moe_run(x, wg, bg, w1, b1, w2, b2, trace=False, trace_kwargs=None):
    x = np.ascontiguousarray(np.asarray(x, np.float32))
    wg = np.asarray(wg, np.float32)
    bg = np.asarray(bg, np.float32)
    w1 = np.asarray(w1, np.float32)
    b1 = np.asarray(b1, np.float32)
    w2 = np.asarray(w2, np.float32)
    b2 = np.asarray(b2, np.float32)
    B = x.shape[0]

    idxs, gs = _route(x, wg, bg)
    cap = max(256, -(-max(len(r) for r in idxs) // 128) * 128)

    nc = build_moe(cap)

    in_maps = []
    for e in range(E):
        n = len(idxs[e])
        xe = np.zeros((cap, D), np.float32)
        xe[:n] = x[idxs[e]]
        ge = np.zeros((cap,), np.float32)
        ge[:n] = gs[e]
        in_maps.append({
            "xt": np.ascontiguousarray(xe.T).astype(np.float16),
            "w1": w1[e].astype(np.float16),
            "w2": w2[e].astype(np.float16),
            "b1": np.ascontiguousarray(b1[e].reshape(NH, 128).T),
            "g": np.ascontiguousarray(np.broadcast_to(ge, (128, cap))),
        })

    kwargs = {}
    if trace:
        kwargs["trace"] = True
        if trace_kwargs:
            kwargs.update(trace_kwargs)
    res = run_bass_kernel_spmd(nc, in_maps, core_ids=list(range(N_CORES)), **kwargs)

    out = np.zeros((B, D), np.float32)
    for e in range(E):
        n = len(idxs[e])
        y = res.results[e]["yt"][:, :n].T  # (n, D), gate already applied
        out[idxs[e]] += y + gs[e][:, None] * b2[e][None, :]
    return out, res


def kernel(x, wg, bg, w1, b1, w2, b2):
    out, _ = moe_run(x, wg, bg, w1, b1, w2, b2, trace=False)
    return out


# revision 12
# speedup vs baseline: 1.3095x; 1.0308x over previous
"""MoE layer (top-2 routing, 8 experts) on 8 Trainium2 NeuronCores.

Strategy (expert-parallel, per sharding hint):
  - Host computes the gate (replicated router math in fp64 numpy): logits,
    top-2 experts per token, softmax gates.
  - Tokens are dispatched (host-side all-to-all) into per-expert batches,
    padded to a common capacity; core c holds expert c's weights and its
    token batch.
  - Each core runs the expert MLP: Y^T = g ⊙ (relu(W1^T X^T + b1) W2)^T in
    a feature-major (transposed) dataflow, bf16 matmuls with fp32 PSUM
    accumulation.
  - Host combines: out[tok] += Y rows (+ g * b2), summing each token's two
    expert contributions.

Hardcoded problem shape: x(8192,1024) w1(8,1024,4096) w2(8,4096,1024).
"""

import numpy as np
import ml_dtypes

import concourse.bass as bass  # noqa: F401  (bass types referenced via tile/bacc)
import concourse.tile as tile
import concourse.mybir as mybir
from concourse import bacc
from concourse.bass_utils import run_bass_kernel_spmd

E = 8          # experts == cores
D = 1024       # model dim
H = 4096       # hidden dim
TOP_K = 2
N_CORES = 8
ND = D // 128  # 8 d-tiles
NH = H // 128  # 32 h-tiles

F32 = mybir.dt.float32
F16 = mybir.dt.float16


def _token_tiles(cap):
    tiles = []
    t = 0
    while t < cap:
        n = min(512, cap - t)
        tiles.append((t, n))
        t += n
    return tiles


def build_moe(cap):
    """Build + compile the per-core expert-MLP Bass program for capacity cap."""
    nc = bacc.Bacc("TRN2", target_bir_lowering=False, debug=False, num_devices=N_CORES)

    xt = nc.dram_tensor("xt", [D, cap], F16, kind="ExternalInput")      # x[idx].T
    w1 = nc.dram_tensor("w1", [D, H], F16, kind="ExternalInput")
    w2 = nc.dram_tensor("w2", [H, D], F16, kind="ExternalInput")
    b1 = nc.dram_tensor("b1", [128, NH], F32, kind="ExternalInput")      # b1[p,j]=b1_full[j*128+p]
    g = nc.dram_tensor("g", [128, cap], F32, kind="ExternalInput")       # gate, replicated rows
    yt = nc.dram_tensor("yt", [D, cap], F32, kind="ExternalOutput")

    xt_ap, w1_ap, w2_ap, b1_ap, g_ap, yt_ap = (
        t.ap() for t in (xt, w1, w2, b1, g, yt)
    )

    with tile.TileContext(nc) as tc:
        with (
            tc.tile_pool(name="wpool", bufs=1) as wpool,
            tc.tile_pool(name="xpool", bufs=2) as xpool,
            tc.tile_pool(name="hpool", bufs=36) as hpool,
            tc.tile_pool(name="ypool", bufs=4) as ypool,
            tc.tile_pool(name="gpool", bufs=2) as gpool,
            tc.tile_pool(name="ph", bufs=4, space="PSUM") as ph_pool,
            tc.tile_pool(name="py", bufs=2, space="PSUM") as py_pool,
        ):
            def load_tok_tile(t0, tn):
                g_sb = gpool.tile([128, 512], F32, name=f"gsb{t0}", tag="gsb")
                nc.sync.dma_start(g_sb[:, :tn], g_ap[:, t0:t0 + tn])
                # One DMA for all 8 d-slices of this token tile: d-slice j
                # lands at columns [j*tn, (j+1)*tn) of a single wide tile.
                xtile = xpool.tile([128, ND * 512], F16, name=f"xsb{t0}", tag="xsb")
                src = xt_ap[:, t0:t0 + tn].rearrange("(dd p) t -> p dd t", p=128)
                dst = xtile[:, :ND * tn].rearrange("p (dd t) -> p dd t", t=tn)
                nc.sync.dma_start(dst, src)
                x_sb = [xtile[:, d * tn:(d + 1) * tn] for d in range(ND)]
                return g_sb, x_sb

            tiles = _token_tiles(cap)
            # Prefetch tile 0's activations BEFORE the bulk weight DMAs so the
            # first layer-1 matmuls aren't queued behind 16.8MB of weights.
            prefetched = {tiles[0][0]: load_tok_tile(*tiles[0])}

            # Resident weights. w1 is loaded as [128, 1024] chunks (8 h-tiles
            # per chunk, 2KB DMA lines) in layer-1 consumption order so tile
            # 0's first matmul group only waits on ~2MB, not all of w1.
            b1_sb = wpool.tile([128, NH], F32, name="b1sb", tag="b1sb")
            nc.sync.dma_start(b1_sb[:], b1_ap[:, :])
            HC = 1024
            n_hc = H // HC
            w1_sb = [[None] * n_hc for _ in range(ND)]
            for hc in range(n_hc):
                for d in range(ND):
                    t = wpool.tile([128, HC], F16, name=f"w1c{d}_{hc}", tag=f"w1c{d}_{hc}")
                    nc.sync.dma_start(
                        t[:], w1_ap[d * 128:(d + 1) * 128, hc * HC:(hc + 1) * HC]
                    )
                    w1_sb[d][hc] = t
            # w2 packed 4 h-slices per tile, triggered on the (otherwise idle)
            # Scalar queue so the Sync queue's serial trigger issue stays short.
            w2_sb = []
            for q in range(NH // 4):
                t = wpool.tile([128, 4 * D], F16, name=f"w2p{q}", tag=f"w2p{q}")
                src = w2_ap[q * 512:(q + 1) * 512, :].rearrange("(ho p) d -> p ho d", p=128)
                dst = t.rearrange("p (ho d) -> p ho d", d=D)
                nc.scalar.dma_start(dst, src)
                w2_sb.append(t)

            for (t0, tn) in tiles:
                g_sb, x_sb = prefetched.pop(t0) if t0 in prefetched else load_tok_tile(t0, tn)

                # Layer 1: H^T[h_tile] = relu(sum_d W1[d,h]^T X^T[d] + b1)
                h_sb = []
                for h in range(NH):
                    ph = ph_pool.tile([128, 512], F32, name=f"ph{t0}_{h}", tag="ph")
                    hc, ho = divmod(h * 128, HC)
                    for d in range(ND):
                        nc.tensor.matmul(
                            ph[:, :tn],
                            w1_sb[d][hc][:, ho:ho + 128],
                            x_sb[d][:, :tn],
                            start=(d == 0),
                            stop=(d == ND - 1),
                        )
                    ht = hpool.tile([128, 512], F16, name=f"hsb{t0}_{h}", tag="hsb")
                    # relu(psum + b1) on DVE (pure ALU — no activation-table
                    # load dependency, unlike ScalarE's ACTIVATE).
                    nc.vector.tensor_scalar(
                        ht[:, :tn], ph[:, :tn],
                        b1_sb[:, h:h + 1], 0.0,
                        op0=mybir.AluOpType.add, op1=mybir.AluOpType.max,
                    )
                    h_sb.append(ht)

                # Layer 2: Y^T[do] = g ⊙ sum_h W2[h,do]^T H^T[h]
                for do in range(ND):
                    py = py_pool.tile([128, 512], F32, name=f"py{t0}_{do}", tag="py")
                    for h in range(NH):
                        q, ho = divmod(h, 4)
                        nc.tensor.matmul(
                            py[:, :tn],
                            w2_sb[q][:, ho * D + do * 128:ho * D + (do + 1) * 128],
                            h_sb[h][:, :tn],
                            start=(h == 0),
                            stop=(h == NH - 1),
                        )
                    y_sb = ypool.tile([128, 512], F32, name=f"ysb{t0}_{do}", tag="ysb")
                    nc.vector.tensor_mul(y_sb[:, :tn], py[:, :tn], g_sb[:, :tn])
                    nc.sync.dma_start(yt_ap[do * 128:(do + 1) * 128, t0:t0 + tn], y_sb[:, :tn])

    nc.compile()
    return nc


def _route(x, wg, bg):
    """Host router in fp64: per-token top-2 experts and softmax gates."""
    logits = x.astype(np.float64) @ wg.astype(np.float64).T + bg.astype(np.float64)
    top2 = np.argpartition(-logits, 1, axis=1)[:, :TOP_K]  # two largest, unordered
    vals = np.take_along_axis(logits, top2, axis=1)
    ex = np.exp(vals - vals.max(axis=1, keepdims=True))
    gates = ex / ex.sum(axis=1, keepdims=True)
    idxs, gs = [], []
    for e in range(E):
        mask = top2 == e
        rows = np.nonzero(mask.any(axis=1))[0]
        idxs.append(rows)
        gs.append(gates[mask].astype(np.float32))
    return idxs, gs


def moe_run(x, wg, bg, w1, b1, w2, b2, trace=False, trace_kwargs=None):
    x = np.ascontiguousarray(np.asarray(x, np.float32))
    wg = np.asarray(wg, np.float32)
    bg = np.asarray(bg, np.float32)
    w1 = np.asarray(w1, np.float32)
    b1 = np.asarray(b1, np.float32)
    w2 = np.asarray(w2, np.float32)
    b2 = np.asarray(b2, np.float32)
    B = x.shape[0]

    idxs, gs = _route(x, wg, bg)
    cap = max(256, -(-max(len(r) for r in idxs) // 128) * 128)

    nc = build_moe(cap)

    in_maps = []
    for e in range(E):
        n = len(idxs[e])
        xe = np.zeros((cap, D), np.float32)
        xe[:n] = x[idxs[e]]
        ge = np.zeros((cap,), np.float32)
        ge[:n] = gs[e]
        in_maps.append({
            "xt": np.ascontiguousarray(xe.T).astype(np.float16),
            "w1": w1[e].astype(np.float16),
            "w2": w2[e].astype(np.float16),
            "b1": np.ascontiguousarray(b1[e].reshape(NH, 128).T),
            "g": np.ascontiguousarray(np.broadcast_to(ge, (128, cap))),
        })

    kwargs = {}
    if trace:
        kwargs["trace"] = True
        if trace_kwargs:
            kwargs.update(trace_kwargs)
    res = run_bass_kernel_spmd(nc, in_maps, core_ids=list(range(N_CORES)), **kwargs)

    out = np.zeros((B, D), np.float32)
    for e in range(E):
        n = len(idxs[e])
        y = res.results[e]["yt"][:, :n].T  # (n, D), gate already applied
        out[idxs[e]] += y + gs[e][:, None] * b2[e][None, :]
    return out, res


def kernel(x, wg, bg, w1, b1, w2, b2):
    out, _ = moe_run(x, wg, bg, w1, b1, w2, b2, trace=False)
    return out


# revision 15
# speedup vs baseline: 1.3334x; 1.0182x over previous
"""MoE layer (top-2 routing, 8 experts) on 8 Trainium2 NeuronCores.

Strategy (expert-parallel, per sharding hint):
  - Host computes the gate (replicated router math in fp64 numpy): logits,
    top-2 experts per token, softmax gates.
  - Tokens are dispatched (host-side all-to-all) into per-expert batches,
    padded to a common capacity; core c holds expert c's weights and its
    token batch.
  - Each core runs the expert MLP: Y^T = g ⊙ (relu(W1^T X^T + b1) W2)^T in
    a feature-major (transposed) dataflow, bf16 matmuls with fp32 PSUM
    accumulation.
  - Host combines: out[tok] += Y rows (+ g * b2), summing each token's two
    expert contributions.

Hardcoded problem shape: x(8192,1024) w1(8,1024,4096) w2(8,4096,1024).
"""

import numpy as np
import ml_dtypes

import concourse.bass as bass  # noqa: F401  (bass types referenced via tile/bacc)
import concourse.tile as tile
import concourse.mybir as mybir
from concourse import bacc
from concourse.bass_utils import run_bass_kernel_spmd

E = 8          # experts == cores
D = 1024       # model dim
H = 4096       # hidden dim
TOP_K = 2
N_CORES = 8
ND = D // 128  # 8 d-tiles
NH = H // 128  # 32 h-tiles

F32 = mybir.dt.float32
F16 = mybir.dt.float16


def _token_tiles(cap):
    tiles = []
    t = 0
    while t < cap:
        n = min(512, cap - t)
        tiles.append((t, n))
        t += n
    return tiles


def build_moe(cap):
    """Build + compile the per-core expert-MLP Bass program for capacity cap."""
    nc = bacc.Bacc("TRN2", target_bir_lowering=False, debug=False, num_devices=N_CORES)

    xt = nc.dram_tensor("xt", [D, cap], F16, kind="ExternalInput")      # x[idx].T
    w1 = nc.dram_tensor("w1", [D, H], F16, kind="ExternalInput")
    w2 = nc.dram_tensor("w2", [H, D], F16, kind="ExternalInput")
    b1 = nc.dram_tensor("b1", [128, NH], F32, kind="ExternalInput")      # b1[p,j]=b1_full[j*128+p]
    g = nc.dram_tensor("g", [128, cap], F32, kind="ExternalInput")       # gate, replicated rows
    yt = nc.dram_tensor("yt", [D, cap], F32, kind="ExternalOutput")

    xt_ap, w1_ap, w2_ap, b1_ap, g_ap, yt_ap = (
        t.ap() for t in (xt, w1, w2, b1, g, yt)
    )

    with tile.TileContext(nc) as tc:
        with (
            tc.tile_pool(name="wpool", bufs=1) as wpool,
            tc.tile_pool(name="xpool", bufs=2) as xpool,
            tc.tile_pool(name="hpool", bufs=36) as hpool,
            tc.tile_pool(name="ypool", bufs=4) as ypool,
            tc.tile_pool(name="gpool", bufs=2) as gpool,
            tc.tile_pool(name="ph", bufs=4, space="PSUM") as ph_pool,
            tc.tile_pool(name="py", bufs=2, space="PSUM") as py_pool,
        ):
            def load_tok_tile(t0, tn, split_first=False):
                # One DMA for all 8 d-slices of this token tile: d-slice j
                # lands at columns [j*tn, (j+1)*tn) of a single wide tile.
                # (split_first: d0 gets its own small DMA so tile 0's first
                # matmul group starts as early as possible.)
                xtile = xpool.tile([128, ND * 512], F16, name=f"xsb{t0}", tag="xsb")
                if split_first:
                    nc.sync.dma_start(xtile[:, :tn], xt_ap[0:128, t0:t0 + tn])
                    src = xt_ap[128:, t0:t0 + tn].rearrange("(dd p) t -> p dd t", p=128)
                    dst = xtile[:, tn:ND * tn].rearrange("p (dd t) -> p dd t", t=tn)
                    nc.sync.dma_start(dst, src)
                else:
                    src = xt_ap[:, t0:t0 + tn].rearrange("(dd p) t -> p dd t", p=128)
                    dst = xtile[:, :ND * tn].rearrange("p (dd t) -> p dd t", t=tn)
                    nc.sync.dma_start(dst, src)
                x_sb = [xtile[:, d * tn:(d + 1) * tn] for d in range(ND)]
                g_sb = gpool.tile([128, 512], F32, name=f"gsb{t0}", tag="gsb")
                nc.sync.dma_start(g_sb[:, :tn], g_ap[:, t0:t0 + tn])
                return g_sb, x_sb

            # PE warm-up: ~20 dummy matmuls on a zeroed tile keep the PE busy
            # during the initial DMA wait so HAM un-throttles before the real
            # stream begins.
            warm = wpool.tile([128, 512], F16, name="warm", tag="warm")
            nc.vector.memset(warm[:], 0.0)
            warm_ps = ph_pool.tile([128, 512], F32, name="warmps", tag="ph")
            for _ in range(20):
                nc.tensor.matmul(warm_ps[:], warm[:, :128], warm[:], start=True, stop=True)

            tiles = _token_tiles(cap)
            # Prefetch tile 0's activations BEFORE the bulk weight DMAs so the
            # first layer-1 matmuls aren't queued behind 16.8MB of weights.
            prefetched = {tiles[0][0]: load_tok_tile(*tiles[0], split_first=True)}

            # Resident weights. w1 is loaded as [128, 1024] chunks (8 h-tiles
            # per chunk, 2KB DMA lines) in layer-1 consumption order so tile
            # 0's first matmul group only waits on ~2MB, not all of w1.
            b1_sb = wpool.tile([128, NH], F32, name="b1sb", tag="b1sb")
            nc.sync.dma_start(b1_sb[:], b1_ap[:, :])
            HC = 1024
            n_hc = H // HC
            w1_sb = [[None] * n_hc for _ in range(ND)]
            for hc in range(n_hc):
                for d in range(ND):
                    t = wpool.tile([128, HC], F16, name=f"w1c{d}_{hc}", tag=f"w1c{d}_{hc}")
                    nc.sync.dma_start(
                        t[:], w1_ap[d * 128:(d + 1) * 128, hc * HC:(hc + 1) * HC]
                    )
                    w1_sb[d][hc] = t
            # w2 packed 4 h-slices per tile, triggered on the (otherwise idle)
            # Scalar queue so the Sync queue's serial trigger issue stays short.
            # The DMAs are dep-gated on tile 0's first layer-1 evac so the
            # 8.4MB w2 stream doesn't steal HBM bandwidth from the startup-
            # critical x0/w1 loads (w2 isn't needed until layer 2, ~55µs in).
            w2_sb = []
            w2_dmas = []
            for q in range(NH // 4):
                t = wpool.tile([128, 4 * D], F16, name=f"w2p{q}", tag=f"w2p{q}")
                src = w2_ap[q * 512:(q + 1) * 512, :].rearrange("(ho p) d -> p ho d", p=128)
                dst = t.rearrange("p (ho d) -> p ho d", d=D)
                w2_dmas.append(nc.scalar.dma_start(dst, src))
                w2_sb.append(t)

            for (t0, tn) in tiles:
                g_sb, x_sb = prefetched.pop(t0) if t0 in prefetched else load_tok_tile(t0, tn)

                # Layer 1: H^T[h_tile] = relu(sum_d W1[d,h]^T X^T[d] + b1)
                h_sb = []
                for h in range(NH):
                    ph = ph_pool.tile([128, 512], F32, name=f"ph{t0}_{h}", tag="ph")
                    hc, ho = divmod(h * 128, HC)
                    for d in range(ND):
                        nc.tensor.matmul(
                            ph[:, :tn],
                            w1_sb[d][hc][:, ho:ho + 128],
                            x_sb[d][:, :tn],
                            start=(d == 0),
                            stop=(d == ND - 1),
                        )
                    ht = hpool.tile([128, 512], F16, name=f"hsb{t0}_{h}", tag="hsb")
                    # relu(psum + b1) on DVE (pure ALU — no activation-table
                    # load dependency, unlike ScalarE's ACTIVATE).
                    evac = nc.vector.tensor_scalar(
                        ht[:, :tn], ph[:, :tn],
                        b1_sb[:, h:h + 1], 0.0,
                        op0=mybir.AluOpType.add, op1=mybir.AluOpType.max,
                    )
                    if t0 == 0 and h == 0:
                        for wd in w2_dmas:
                            tile.add_dep_helper(wd.ins, evac.ins, sync=True,
                                                reason="w2 prefetch after startup-critical loads")
                    h_sb.append(ht)

                # Layer 2: Y^T[do] = g ⊙ sum_h W2[h,do]^T H^T[h]
                for do in range(ND):
                    py = py_pool.tile([128, 512], F32, name=f"py{t0}_{do}", tag="py")
                    for h in range(NH):
                        q, ho = divmod(h, 4)
                        nc.tensor.matmul(
                            py[:, :tn],
                            w2_sb[q][:, ho * D + do * 128:ho * D + (do + 1) * 128],
                            h_sb[h][:, :tn],
                            start=(h == 0),
                            stop=(h == NH - 1),
                        )
                    y_sb = ypool.tile([128, 512], F32, name=f"ysb{t0}_{do}", tag="ysb")
                    nc.vector.tensor_mul(y_sb[:, :tn], py[:, :tn], g_sb[:, :tn])
                    nc.sync.dma_start(yt_ap[do * 128:(do + 1) * 128, t0:t0 + tn], y_sb[:, :tn])

    nc.compile()
    return nc


def _route(x, wg, bg):
    """Host router in fp64: per-token top-2 experts and softmax gates."""
    logits = x.astype(np.float64) @ wg.astype(np.float64).T + bg.astype(np.float64)
    top2 = np.argpartition(-logits, 1, axis=1)[:, :TOP_K]  # two largest, unordered
    vals = np.take_along_axis(logits, top2, axis=1)
    ex = np.exp(vals - vals.max(axis=1, keepdims=True))
    gates = ex / ex.sum(axis=1, keepdims=True)
    idxs, gs = [], []
    for e in range(E):
        mask = top2 == e
        rows = np.nonzero(mask.any(axis=1))[0]
        idxs.append(rows)
        gs.append(gates[mask].astype(np.float32))
    return idxs, gs


def moe_run(x, wg, bg, w1, b1, w2, b2, trace=False, trace_kwargs=None):
    x = np.ascontiguousarray(np.asarray(x, np.float32))
    wg = np.asarray(wg, np.float32)
    bg = np.asarray(bg, np.float32)
    w1 = np.asarray(w1, np.float32)
    b1 = np.asarray(b1, np.float32)
    w2 = np.asarray(w2, np.float32)
    b2 = np.asarray(b2, np.float32)
    B = x.shape[0]

    idxs, gs = _route(x, wg, bg)
    cap = max(256, -(-max(len(r) for r in idxs) // 128) * 128)

    nc = build_moe(cap)

    in_maps = []
    for e in range(E):
        n = len(idxs[e])
        xe = np.zeros((cap, D), np.float32)
        xe[:n] = x[idxs[e]]
        ge = np.zeros((cap,), np.float32)
        ge[:n] = gs[e]
        in_maps.append({
            "xt": np.ascontiguousarray(xe.T).astype(np.float16),
            "w1": w1[e].astype(np.float16),
            "w2": w2[e].astype(np.float16),
            "b1": np.ascontiguousarray(b1[e].reshape(NH, 128).T),
            "g": np.ascontiguousarray(np.broadcast_to(ge, (128, cap))),
        })

    kwargs = {}
    if trace:
        kwargs["trace"] = True
        if trace_kwargs:
            kwargs.update(trace_kwargs)
    res = run_bass_kernel_spmd(nc, in_maps, core_ids=list(range(N_CORES)), **kwargs)

    out = np.zeros((B, D), np.float32)
    for e in range(E):
        n = len(idxs[e])
        y = res.results[e]["yt"][:, :n].T  # (n, D), gate already applied
        out[idxs[e]] += y + gs[e][:, None] * b2[e][None, :]
    return out, res


def kernel(x, wg, bg, w1, b1, w2, b2):
    out, _ = moe_run(x, wg, bg, w1, b1, w2, b2, trace=False)
    return out
